# revision 29
# baseline (speedup 1.0000x reference)
"""Trainium2 Bass kernel for nn_Encoder (dense transformer encoder layer).

Sharding: 8 NeuronCores, sequence-parallel, zero collectives. B*S = 4096
rows -> 512 rows per core; cores 0-3 handle batch 0, cores 4-7 batch 1.
The full batch's x (transposed, bf16) is replicated to every core as an
input, so each core computes K^T and V for the WHOLE batch locally (no
K/V AllGather), then runs attention for its own 512 query rows over all
16 heads, plus Wo / LN1 / FFN / LN2 locally.

All matmuls bf16 (full PE rate, weights pre-cast host-side); psum
accumulation fp32. End-to-end relative error ~3e-3 (budget 2e-2).

Schedule: K/V/Q projections for head-pair p+2 are woven between the
attention units of pair p (thunk queue), so the PE stays dense while
the scalar engine streams the softmax exp() ops.

Dataflow (feature-on-partition):
  xT [8dc x 128, 2048] bf16 (host-transposed full-batch input)
  kT[p] [128, 2048] = Wk_p.T @ xT  (full batch keys)
  vq[q]: V columns for pairs 2q,2q+1, keys on partition, ones column
         appended per (pair,hh,keytile) for the softmax denominator
  qT[p] [128, 512] from xoT (own rows, host-transposed)
  S^T [128 keys, 512 q] = kts.T @ qT  (2 heads packed at rows 0/64)
  P = exp(0.125 * S^T) -> bf16, one ACT op per 2 key tiles
  O^T[65,512] += [V|1].T @ P  (psum row 64 = denominator)
  oT = O^T * bcast(1/den) + bv;  mhaT = Wo.T @ oT (+bo)
  x1 = transpose(mhaT) + x -> LN1 -> x1n -> x1nT
  hT = relu(W1.T @ x1nT + b1); ffnT = W2.T @ hT
  x2T = ffnT + b2 + x1nT -> transpose -> LN2 -> out [512, 1024] f32
"""

from collections import deque

import numpy as np

import concourse.bass as bass
import concourse.mybir as mybir
from concourse import bacc
from concourse.tile import TileContext
from concourse.bass_utils import run_bass_kernel_spmd

F32 = mybir.dt.float32
BF = mybir.dt.bfloat16
AF = mybir.ActivationFunctionType
OP = mybir.AluOpType

B, S, D = 2, 2048, 1024
H, DK, DFF = 16, 64, 4096
NCORES = 8
RPC = S * B // NCORES          # 512 own rows per core
FB = S                         # 2048 full-batch rows
NP = H // 2                    # 8 head pairs
NKT = FB // 128                # 16 key tiles

_TCNT = [0]


def _mk(pool, shape, dt, tag, bufs=None):
    _TCNT[0] += 1
    kw = {} if bufs is None else {"bufs": bufs}
    return pool.tile(shape, dt, tag=tag, name=f"t{_TCNT[0]}_{tag}", **kw)


def build_nc():
    nc = bacc.Bacc(num_devices=NCORES)

    ins = dict(
        xbT=nc.dram_tensor("xbT", [D, FB], BF, kind="ExternalInput"),
        xoT=nc.dram_tensor("xoT", [D, RPC], BF, kind="ExternalInput"),
        xn=nc.dram_tensor("xn", [RPC, D], BF, kind="ExternalInput"),
        wq=nc.dram_tensor("wq", [NP, 128, 1024], BF, kind="ExternalInput"),
        wk=nc.dram_tensor("wk", [NP, 128, 1024], BF, kind="ExternalInput"),
        wv=nc.dram_tensor("wv", [2, 128, 4096], BF, kind="ExternalInput"),
        wo=nc.dram_tensor("wo", [8, 128, 1024], BF, kind="ExternalInput"),
        w1=nc.dram_tensor("w1", [32, 128, 1024], BF, kind="ExternalInput"),
        w2=nc.dram_tensor("w2", [32, 128, 1024], BF, kind="ExternalInput"),
        bq=nc.dram_tensor("bq", [D, 1], F32, kind="ExternalInput"),
        bk=nc.dram_tensor("bk", [D, 1], F32, kind="ExternalInput"),
        bv=nc.dram_tensor("bv", [D, 1], F32, kind="ExternalInput"),
        bvr=nc.dram_tensor("bvr", [1, D], BF, kind="ExternalInput"),
        bo=nc.dram_tensor("bo", [D, 1], F32, kind="ExternalInput"),
        b1=nc.dram_tensor("b1", [DFF, 1], F32, kind="ExternalInput"),
        b2=nc.dram_tensor("b2", [D, 1], F32, kind="ExternalInput"),
        g1=nc.dram_tensor("g1", [1, D], BF, kind="ExternalInput"),
        be1=nc.dram_tensor("be1", [1, D], BF, kind="ExternalInput"),
        g2=nc.dram_tensor("g2", [1, D], BF, kind="ExternalInput"),
        be2=nc.dram_tensor("be2", [1, D], BF, kind="ExternalInput"),
        ident=nc.dram_tensor("ident", [128, 128], BF, kind="ExternalInput"),
        out=nc.dram_tensor("out", [RPC, D], F32, kind="ExternalOutput"),
    )

    with TileContext(nc) as tc:
        _body(nc, tc, ins)

    nc.finalize()
    return nc


def _body(nc, tc, ins):
    with (
        tc.tile_pool(name="outer", bufs=1) as po,
        tc.tile_pool(name="psum", bufs=1, space="PSUM") as pp,
    ):
        # ---- constants ----
        ident = _mk(po, [128, 128], BF, "ident")
        nc.scalar.dma_start(out=ident[:], in_=ins["ident"][:])
        bias = {}
        for nm, n in (("bq", 8), ("bk", 8), ("bo", 8), ("b1", 32), ("b2", 8)):
            t = _mk(po, [128, n], F32, "b_" + nm)
            nc.scalar.dma_start(out=t[:],
                                in_=ins[nm].rearrange("(i p) o -> p (i o)", p=128))
            bias[nm] = t
        # bv in per-head layout: col h = bv[h*64:(h+1)*64] on partitions 0-63
        bvh = _mk(po, [128, H], F32, "bvh")
        nc.scalar.dma_start(out=bvh[0:64, :],
                            in_=ins["bv"].rearrange("(h p) o -> p (h o)", p=64))
        bvr = _mk(po, [128, D], BF, "bvr")
        nc.scalar.dma_start(out=bvr[:], in_=ins["bvr"].broadcast_to([128, D]))
        lnw = {}
        for nm in ("g1", "be1", "g2", "be2"):
            t = _mk(po, [128, D], BF, "ln_" + nm)
            nc.scalar.dma_start(out=t[:], in_=ins[nm].broadcast_to([128, D]))
            lnw[nm] = t
        eps = _mk(po, [128, 1], F32, "eps")
        nc.vector.memset(eps[:], 1e-5)
        lnw["eps"] = eps
        ones_f = _mk(po, [128, 64], F32, "ones_f")
        nc.vector.memset(ones_f[:], 1.0)
        lnw["ones_f"] = ones_f
        xn = [_mk(po, [128, D], BF, f"xn{r}") for r in range(4)]
        for r in range(4):
            nc.scalar.dma_start(out=xn[r][:],
                                in_=ins["xn"][r * 128:(r + 1) * 128, :])

        # persistent post-phase activations
        mhaT = [_mk(po, [128, RPC], BF, f"mhaT{oc}") for oc in range(8)]
        x1nT = [_mk(po, [128, RPC], BF, f"x1nT{dc}") for dc in range(8)]

        with tc.tile_pool(name="attn", bufs=1) as pa:
            _attn_phase(nc, tc, ins, po, pa, pp, ident, bias, bvh, bvr,
                        xn, mhaT, x1nT, lnw)

        with tc.tile_pool(name="post", bufs=1) as pf:
            _post_phase(nc, tc, ins, pf, pp, ident, bias, lnw, x1nT)


def _attn_phase(nc, tc, ins, po, pa, pp, ident, bias, bvh, bvr,
                xn, mhaT, x1nT, lnw):
    oT = [_mk(pa, [128, RPC], BF, f"oT{p}") for p in range(NP)]
    kTs, qTs, vqs = {}, {}, {}

    with tc.tile_pool(name="proj", bufs=1) as px:
        xT = [_mk(px, [128, FB], BF, f"xT{dc}") for dc in range(8)]
        for dc in range(8):
            nc.sync.dma_start(out=xT[dc][:],
                              in_=ins["xbT"][dc * 128:(dc + 1) * 128, :])
        xoT = [_mk(px, [128, RPC], BF, f"xoT{dc}") for dc in range(8)]
        for dc in range(8):
            nc.scalar.dma_start(out=xoT[dc][:],
                                in_=ins["xoT"][dc * 128:(dc + 1) * 128, :])

        def proj_thunks(p):
            """Thunk list computing kT/qT (and V quarter when p even)."""
            thunks = []
            st = {}

            def dma_w():
                wkt = _mk(px, [128, 1024], BF, "wkt", bufs=2)
                nc.sync.dma_start(out=wkt[:], in_=ins["wk"][p])
                wqt = _mk(px, [128, 1024], BF, "wqt", bufs=2)
                nc.sync.dma_start(out=wqt[:], in_=ins["wq"][p])
                kTs[p] = _mk(pa, [128, FB], BF, "kT", bufs=3)
                qTs[p] = _mk(pa, [128, RPC], BF, "qT", bufs=3)
                st["wkt"], st["wqt"] = wkt, wqt

            thunks.append(dma_w)

            def k_chunk(c):
                ps = _mk(pp, [128, 512], F32, "ps_p", bufs=2)
                for dc in range(8):
                    nc.tensor.matmul(ps[:],
                                     st["wkt"][:, dc * 128:(dc + 1) * 128],
                                     xT[dc][:, c * 512:(c + 1) * 512],
                                     start=(dc == 0), stop=(dc == 7),
                                     skip_group_check=True)
                nc.vector.tensor_scalar(kTs[p][:, c * 512:(c + 1) * 512],
                                        ps[:], bias["bk"][:, p:p + 1],
                                        None, OP.add)

            for c in range(4):
                thunks.append(lambda c=c: k_chunk(c))

            def q_chunk():
                ps = _mk(pp, [128, 512], F32, "ps_p", bufs=2)
                for dc in range(8):
                    nc.tensor.matmul(ps[:],
                                     st["wqt"][:, dc * 128:(dc + 1) * 128],
                                     xoT[dc][:], start=(dc == 0), stop=(dc == 7),
                                     skip_group_check=True)
                nc.vector.tensor_scalar(qTs[p][:], ps[:],
                                        bias["bq"][:, p:p + 1], None, OP.add)

            thunks.append(q_chunk)

            if p % 4 == 0:
                hf = p // 4

                def dma_v():
                    wvt = _mk(px, [128, 4096], BF, "wvt", bufs=1)
                    nc.sync.dma_start(out=wvt[:], in_=ins["wv"][hf])
                    vq = _mk(pa, [128, 4 * 2 * NKT * 65], BF, "vq", bufs=2)
                    vqs[hf] = vq
                    vqv = vq[:].rearrange("k (i h t c) -> k i h t c",
                                          i=4, h=2, c=65)
                    for pl in range(4):
                        nc.vector.memset(vqv[:, pl, :, :, 64:65], 1.0)
                    st["wvt"] = wvt

                thunks.append(dma_v)

                def v_chunk(kt2):
                    vqv = vqs[hf][:].rearrange("k (i h t c) -> k i h t c",
                                               i=4, h=2, c=65)
                    bvs = bvr[:, hf * 512:(hf + 1) * 512].rearrange(
                        "k (i h c) -> k i h c", i=4, c=64)
                    ps = _mk(pp, [128, 512], F32, "ps_p", bufs=2)
                    for dc in range(8):
                        nc.tensor.matmul(ps[:],
                                         xT[dc][:, kt2 * 128:(kt2 + 1) * 128],
                                         st["wvt"][:, dc * 512:(dc + 1) * 512],
                                         start=(dc == 0), stop=(dc == 7),
                                         skip_group_check=True)
                    nc.vector.tensor_tensor(
                        vqv[:, :, :, kt2, 0:64],
                        ps[:].rearrange("k (i h c) -> k i h c", i=4, c=64),
                        bvs, OP.add)

                for kt2 in range(NKT):
                    thunks.append(lambda kt2=kt2: v_chunk(kt2))
            return thunks

        def attn_unit(p, u):
            kt, qt, vq = kTs[p], qTs[p], vqs[p // 4]
            pl = p % 4
            vqv = vq[:].rearrange("k (i h t c) -> k i h t c", i=4, h=2, c=65)
            pss = [_mk(pp, [128, 1024], F32, "ps_s", bufs=2) for _ in range(2)]
            for i in range(2):
                ktile = 2 * u + i
                for hh in range(2):
                    nc.tensor.matmul(
                        pss[hh][:, i * 512:(i + 1) * 512],
                        kt[hh * 64:(hh + 1) * 64,
                           ktile * 128:(ktile + 1) * 128],
                        qt[hh * 64:(hh + 1) * 64, :],
                        start=True, stop=True, skip_group_check=True)
            pts = []
            for hh in range(2):
                pt = _mk(pa, [128, 1024], BF, "pt", bufs=4)
                nc.scalar.activation(pt[:], pss[hh][:], AF.Exp,
                                     bias=0.0, scale=0.125)
                pts.append(pt)
            for i in range(2):
                ktile = 2 * u + i
                for hh in range(2):
                    nc.tensor.matmul(
                        _PSO[hh][0:65, :],
                        vqv[:, pl, hh, ktile, :],
                        pts[hh][:, i * 512:(i + 1) * 512],
                        start=(ktile == 0), stop=(ktile == NKT - 1),
                        skip_group_check=True)

        def attn_norm(p):
            for hh in range(2):
                h = 2 * p + hh
                den = _mk(pa, [128, 512], F32, "den", bufs=2)
                nc.vector.tensor_copy(den[64:65, :], _PSO[hh][64:65, :])
                rden = _mk(pa, [128, 512], F32, "rden", bufs=2)
                nc.vector.reciprocal(rden[64:65, :], den[64:65, :])
                # broadcast 1/den across 64 partitions via the PE array
                ps_b = _mk(pp, [128, 512], F32, "ps_p", bufs=2)
                nc.tensor.matmul(ps_b[0:64, :], lnw["ones_f"][64:65, :],
                                 rden[64:65, :], start=True, stop=True,
                                 skip_group_check=True)
                rb = _mk(pa, [128, 512], F32, "rb", bufs=2)
                nc.vector.tensor_copy(rb[0:64, :], ps_b[0:64, :])
                tmp = _mk(pa, [128, 512], F32, "onorm", bufs=2)
                nc.vector.tensor_tensor(tmp[0:64, :], _PSO[hh][0:64, :],
                                        rb[0:64, :], OP.mult)
                if hh == 0:
                    nc.vector.tensor_scalar(oT[p][0:64, :], tmp[0:64, :],
                                            bvh[0:64, h:h + 1], None, OP.add)
                else:
                    stage = _mk(pa, [128, 512], BF, "stage", bufs=2)
                    nc.vector.tensor_scalar(stage[0:64, :], tmp[0:64, :],
                                            bvh[0:64, h:h + 1], None, OP.add)
                    nc.gpsimd.dma_start(out=oT[p][64:128, :],
                                        in_=stage[0:64, :])

        # ---- software-pipelined schedule: proj runs 2 pairs ahead ----
        pending = deque()
        for t in proj_thunks(0) + proj_thunks(1):
            t()
        for p in range(NP):
            if p + 2 < NP:
                pending.extend(proj_thunks(p + 2))
            _PSO = [_mk(pp, [128, 512], F32, "ps_o", bufs=2) for _ in range(2)]
            for u in range(NKT // 2):
                attn_unit(p, u)
                slots_left = NKT // 2 - u
                k = (len(pending) + slots_left - 1) // slots_left
                for _ in range(min(k, len(pending))):
                    pending.popleft()()
            attn_norm(p)
        assert not pending

    # ---- Wo + residual (back to natural domain) ----
    for oc in range(8):
        wot = _mk(pa, [128, 1024], BF, "wot", bufs=4)
        nc.sync.dma_start(out=wot[:], in_=ins["wo"][oc])
        ps = _mk(pp, [128, 512], F32, "ps_p", bufs=2)
        for dc in range(8):
            nc.tensor.matmul(ps[:], wot[:, dc * 128:(dc + 1) * 128],
                             oT[dc][:], start=(dc == 0), stop=(dc == 7),
                             skip_group_check=True)
        nc.vector.tensor_scalar(mhaT[oc][:], ps[:],
                                bias["bo"][:, oc:oc + 1], None, OP.add)
    # x1 = transpose(mhaT) + x, then LN1 per row-tile (pipelined), with
    # x1n^T transposes woven in so the PE keeps a trickle of work.
    # x1nT psum: 2 tiles [128, 2048] bf16 (tag ps_s byte size), block
    # (dc%4, r) at column (dc%4)*512 + r*128.
    pst = [_mk(pp, [128, 2048], BF, "ps_s", bufs=2) for _ in range(2)]
    for r in range(4):
        ps = _mk(pp, [128, 1024], BF, "ps_p", bufs=2)
        for oc in range(8):
            nc.tensor.transpose(ps[:, oc * 128:(oc + 1) * 128],
                                mhaT[oc][:, r * 128:(r + 1) * 128], ident[:])
        x1 = _mk(po, [128, D], BF, "x1", bufs=2)
        nc.vector.tensor_tensor(x1[:], ps[:], xn[r][:], OP.add)
        x1n = _mk(po, [128, D], BF, "x1n", bufs=2)
        _layernorm(nc, po, x1n, x1, lnw["g1"], lnw["be1"], lnw["eps"])
        for dc in range(8):
            nc.tensor.transpose(
                pst[dc // 4][:, (dc % 4) * 512 + r * 128:
                             (dc % 4) * 512 + (r + 1) * 128],
                x1n[:, dc * 128:(dc + 1) * 128], ident[:])
    for dc in range(8):
        nc.vector.tensor_copy(x1nT[dc][:],
                              pst[dc // 4][:, (dc % 4) * 512:
                                           (dc % 4 + 1) * 512])


def _post_phase(nc, tc, ins, pf, pp, ident, bias, lnw, x1nT):
    # ---- FFN ----
    w2t = [_mk(pf, [128, 1024], BF, f"w2t{f}") for f in range(32)]
    hT = [_mk(pf, [128, RPC], BF, f"hT{f}") for f in range(32)]
    for f in range(32):
        w1t = _mk(pf, [128, 1024], BF, "w1t", bufs=4)
        nc.sync.dma_start(out=w1t[:], in_=ins["w1"][f])
        nc.sync.dma_start(out=w2t[f][:], in_=ins["w2"][f])
        ps = _mk(pp, [128, 512], F32, "ps_p", bufs=2)
        for dc in range(8):
            nc.tensor.matmul(ps[:], w1t[:, dc * 128:(dc + 1) * 128],
                             x1nT[dc][:], start=(dc == 0), stop=(dc == 7),
                             skip_group_check=True)
        nc.scalar.activation(hT[f][:], ps[:], AF.Relu,
                             bias=bias["b1"][:, f:f + 1], scale=1.0)

    x2T = [_mk(pf, [128, RPC], BF, f"x2T{oc}") for oc in range(8)]
    for oc in range(8):
        ps = _mk(pp, [128, 512], F32, "ps_p", bufs=2)
        for f in range(32):
            nc.tensor.matmul(ps[:], w2t[f][:, oc * 128:(oc + 1) * 128],
                             hT[f][:], start=(f == 0), stop=(f == 31),
                             skip_group_check=True)
        tmp = _mk(pf, [128, RPC], BF, "f2tmp", bufs=2)
        nc.vector.tensor_tensor(tmp[:], ps[:], x1nT[oc][:], OP.add)
        nc.vector.tensor_scalar(x2T[oc][:], tmp[:],
                                bias["b2"][:, oc:oc + 1], None, OP.add)

    # ---- transpose back, LN2, out ----
    for r in range(4):
        ps = _mk(pp, [128, 1024], BF, "ps_p", bufs=2)
        for oc in range(8):
            nc.tensor.transpose(ps[:, oc * 128:(oc + 1) * 128],
                                x2T[oc][:, r * 128:(r + 1) * 128], ident[:])
        x2 = _mk(pf, [128, D], BF, "x2", bufs=2)
        nc.vector.tensor_copy(x2[:], ps[:])
        outt = _mk(pf, [128, D], F32, "outt", bufs=2)
        _layernorm(nc, pf, outt, x2, lnw["g2"], lnw["be2"], lnw["eps"])
        nc.sync.dma_start(out=ins["out"][r * 128:(r + 1) * 128, :],
                          in_=outt[:])


def _layernorm(nc, pool, out, x, g, be, eps):
    """LN along the free dim (D=1024). x [128, 1024] bf16; out bf16/f32."""
    _TCNT[0] += 1
    n = _TCNT[0]
    stats = pool.tile([128, 2, 6], F32, tag="ln_st", bufs=2, name=f"lnst{n}")
    for i in range(2):
        nc.vector.bn_stats(stats[:, i, :], x[:, i * 512:(i + 1) * 512])
    mv = pool.tile([128, 2], F32, tag="ln_mv", bufs=2, name=f"lnmv{n}")
    nc.vector.bn_aggr(mv[:], stats[:])
    std = pool.tile([128, 1], F32, tag="ln_sd", bufs=2, name=f"lnsd{n}")
    nc.scalar.activation(std[:], mv[:, 1:2], AF.Sqrt, bias=eps, scale=1.0)
    rstd = pool.tile([128, 1], F32, tag="ln_rs", bufs=2, name=f"lnrs{n}")
    nc.vector.reciprocal(rstd[:], std[:])
    t = pool.tile([128, D], BF, tag="ln_t", bufs=2, name=f"lnt{n}")
    nc.vector.tensor_scalar(t[:], x[:], mv[:, 0:1], rstd[:],
                            OP.subtract, OP.mult)
    t2 = pool.tile([128, D], BF, tag="ln_t2", bufs=2, name=f"lnt2{n}")
    nc.vector.tensor_tensor(t2[:], t[:], g[:], OP.mult)
    nc.vector.tensor_tensor(out[:], t2[:], be[:], OP.add)


def prep_inputs(x, Wq, bq, Wk, bk, Wv, bv, Wo, bo, W1, b1, W2, b2,
                g1, be1, g2, be2):
    """Host-side prep: per-core inputs, weights pre-cast to bf16.

    Stationary-weight layouts are [*, 128, n] with the 128 SBUF
    partitions contiguous-major so each tile is one dense DMA.
    """
    import ml_dtypes
    f = np.float32
    bf = ml_dtypes.bfloat16

    def _qdc(w, ncol):  # [D_in, ncols] -> [ncols/ncol, 128, 8*ncol]
        # element (blk, q, dc*ncol+c) = w[dc*128+q, blk*ncol+c]
        nblk = w.shape[1] // ncol
        return np.ascontiguousarray(
            np.asarray(w, f).reshape(8, 128, nblk, ncol).transpose(2, 1, 0, 3)
            .reshape(nblk, 128, 8 * ncol)).astype(bf)

    wq_flat = np.asarray(Wq, f).transpose(1, 0, 2).reshape(D, D)
    wk_flat = np.asarray(Wk, f).transpose(1, 0, 2).reshape(D, D)
    wv_flat = np.asarray(Wv, f).transpose(1, 0, 2).reshape(D, D)
    common = {
        "wq": _qdc(wq_flat, 128), "wk": _qdc(wk_flat, 128),
        "wv": _qdc(wv_flat, 512), "wo": _qdc(np.asarray(Wo, f), 128),
        "w1": _qdc(np.asarray(W1, f), 128),
        "w2": np.asarray(W2, f).reshape(32, 128, 1024).astype(bf),
        "bq": np.asarray(bq, f).reshape(D, 1),
        "bk": np.asarray(bk, f).reshape(D, 1),
        "bv": np.asarray(bv, f).reshape(D, 1),
        "bvr": np.asarray(bv, f).reshape(1, D).astype(bf),
        "bo": np.asarray(bo, f).reshape(D, 1),
        "b1": np.asarray(b1, f).reshape(DFF, 1),
        "b2": np.asarray(b2, f).reshape(D, 1),
        "g1": np.asarray(g1, f).reshape(1, D).astype(bf),
        "be1": np.asarray(be1, f).reshape(1, D).astype(bf),
        "g2": np.asarray(g2, f).reshape(1, D).astype(bf),
        "be2": np.asarray(be2, f).reshape(1, D).astype(bf),
        "ident": np.eye(128, dtype=f).astype(bf),
    }
    xf = np.asarray(x, f)
    xbT = [np.ascontiguousarray(xf[b].T).astype(bf) for b in range(B)]
    in_maps = []
    for c in range(NCORES):
        b, j = divmod(c, 4)
        m = dict(common)
        m["xbT"] = xbT[b]
        own = xf[b, j * RPC:(j + 1) * RPC, :]
        m["xoT"] = np.ascontiguousarray(own.T).astype(bf)
        m["xn"] = np.ascontiguousarray(own).astype(bf)
        in_maps.append(m)
    return in_maps


_NC_CACHE = {}
LAST_EXEC_NS = None
LAST_TRACE_PATH = None
LAST_PROFILE_JSON = None


def kernel(**inputs) -> np.ndarray:
    global LAST_EXEC_NS, LAST_TRACE_PATH, LAST_PROFILE_JSON
    if "main" not in _NC_CACHE:
        _NC_CACHE["main"] = build_nc()
    nc = _NC_CACHE["main"]
    in_maps = prep_inputs(**inputs)
    res = run_bass_kernel_spmd(nc, in_maps, core_ids=list(range(NCORES)))
    LAST_EXEC_NS = getattr(res, "exec_time_ns", None)
    LAST_PROFILE_JSON = getattr(res, "profile_json", None)
    it = getattr(res, "instructions_and_trace", None)
    LAST_TRACE_PATH = it[1] if it else None
    out = np.empty((B, S, D), np.float32)
    for c in range(NCORES):
        b, j = divmod(c, 4)
        out[b, j * RPC:(j + 1) * RPC, :] = res.results[c]["out"]
    return out


# revision 32
# speedup vs baseline: 1.2246x; 1.2246x over previous
"""Trainium2 Bass kernel for nn_Encoder (dense transformer encoder layer).

Sharding: 8 NeuronCores, sequence-parallel, zero collectives. B*S = 4096
rows -> 512 rows per core; cores 0-3 handle batch 0, cores 4-7 batch 1.
The full batch's x (transposed, bf16) is replicated to every core as an
input, so each core computes K^T and V for the WHOLE batch locally (no
K/V AllGather), then runs attention for its own 512 query rows over all
16 heads, plus Wo / LN1 / FFN / LN2 locally.

All matmuls bf16 (full PE rate, weights pre-cast host-side); psum
accumulation fp32. End-to-end relative error ~1e-2 (budget 2e-2).

Schedule: every projection chunk (K/Q/V) carries a (pair, unit)
deadline; chunks are emitted just-in-time between attention units so
the PE never idles long enough to re-throttle while the scalar engine
streams the softmax exp() ops. Attention starts ~10us into the kernel.

Dataflow:
  xT [8dc x 128, 2048] bf16 (host-transposed full-batch input)
  kT[p] [128, 2048] = Wk_p.T @ xT  (full batch keys, feature-major)
  vq[half]: V columns for 4 pairs, keys on partition, ones column per
            (pair,hh,keytile) for the softmax denominator
  qT[p] [128, 512] from xoT (own rows, host-transposed)
  S^T [128 keys, 512 q] = kts.T @ qT  (2 heads packed at rows 0/64)
  P = exp(0.125 * S^T) -> bf16, one ACT op per 2 key tiles
  O^T[65,512] += [V|1].T @ P  (psum row 64 = denominator)
  oT = O^T * bcast(exp(-ln den)) + bv
  mha (natural) = sum_dc oT_chunk.T @ Wo_rows  (stationary reuse, no
      output transpose); x1 = mha + (x + bo)   [bo folded host-side]
  LN1 -> x1n -> x1nT;  hT = relu(W1.T @ x1nT + b1)
  ffn (natural) = sum_f hT_chunk.T @ W2_rows; x2 = ffn + x1n + b2
  LN2 -> out [512, 1024] f32
"""

from collections import deque

import numpy as np

import concourse.bass as bass
import concourse.mybir as mybir
from concourse import bacc
from concourse.tile import TileContext
from concourse.bass_utils import run_bass_kernel_spmd

F32 = mybir.dt.float32
BF = mybir.dt.bfloat16
AF = mybir.ActivationFunctionType
OP = mybir.AluOpType

B, S, D = 2, 2048, 1024
H, DK, DFF = 16, 64, 4096
NCORES = 8
RPC = S * B // NCORES          # 512 own rows per core
FB = S                         # 2048 full-batch rows
NP = H // 2                    # 8 head pairs
NKT = FB // 128                # 16 key tiles

_TCNT = [0]


def _mk(pool, shape, dt, tag, bufs=None):
    _TCNT[0] += 1
    kw = {} if bufs is None else {"bufs": bufs}
    return pool.tile(shape, dt, tag=tag, name=f"t{_TCNT[0]}_{tag}", **kw)


def build_nc():
    nc = bacc.Bacc(num_devices=NCORES)

    ins = dict(
        xbT=nc.dram_tensor("xbT", [D, FB], BF, kind="ExternalInput"),
        xoT=nc.dram_tensor("xoT", [D, RPC], BF, kind="ExternalInput"),
        xn=nc.dram_tensor("xn", [RPC, D], BF, kind="ExternalInput"),
        wq=nc.dram_tensor("wq", [NP, 128, 1024], BF, kind="ExternalInput"),
        wk=nc.dram_tensor("wk", [NP, 128, 1024], BF, kind="ExternalInput"),
        wv=nc.dram_tensor("wv", [2, 128, 4096], BF, kind="ExternalInput"),
        wo=nc.dram_tensor("wo", [8, 128, 1024], BF, kind="ExternalInput"),
        w1=nc.dram_tensor("w1", [32, 128, 1024], BF, kind="ExternalInput"),
        w2=nc.dram_tensor("w2", [32, 128, 1024], BF, kind="ExternalInput"),
        bq=nc.dram_tensor("bq", [D, 1], F32, kind="ExternalInput"),
        bk=nc.dram_tensor("bk", [D, 1], F32, kind="ExternalInput"),
        bv=nc.dram_tensor("bv", [D, 1], F32, kind="ExternalInput"),
        bvr=nc.dram_tensor("bvr", [1, D], BF, kind="ExternalInput"),
        b1=nc.dram_tensor("b1", [DFF, 1], F32, kind="ExternalInput"),
        b2r=nc.dram_tensor("b2r", [1, D], BF, kind="ExternalInput"),
        g1=nc.dram_tensor("g1", [1, D], BF, kind="ExternalInput"),
        be1=nc.dram_tensor("be1", [1, D], BF, kind="ExternalInput"),
        g2=nc.dram_tensor("g2", [1, D], BF, kind="ExternalInput"),
        be2=nc.dram_tensor("be2", [1, D], BF, kind="ExternalInput"),
        ident=nc.dram_tensor("ident", [128, 128], BF, kind="ExternalInput"),
        out=nc.dram_tensor("out", [RPC, D], F32, kind="ExternalOutput"),
    )

    with TileContext(nc) as tc:
        _body(nc, tc, ins)

    nc.finalize()
    return nc


def _body(nc, tc, ins):
    with (
        tc.tile_pool(name="outer", bufs=1) as po,
        tc.tile_pool(name="psum", bufs=1, space="PSUM") as pp,
    ):
        # ---- constants ----
        ident = _mk(po, [128, 128], BF, "ident")
        nc.scalar.dma_start(out=ident[:], in_=ins["ident"][:])
        bias = {}
        for nm, n in (("bq", 8), ("bk", 8), ("b1", 32)):
            t = _mk(po, [128, n], F32, "b_" + nm)
            nc.scalar.dma_start(out=t[:],
                                in_=ins[nm].rearrange("(i p) o -> p (i o)", p=128))
            bias[nm] = t
        # bv in per-head layout: col h = bv[h*64:(h+1)*64] on partitions 0-63
        bvh = _mk(po, [128, H], F32, "bvh")
        nc.scalar.dma_start(out=bvh[0:64, :],
                            in_=ins["bv"].rearrange("(h p) o -> p (h o)", p=64))
        bvr = _mk(po, [128, D], BF, "bvr")
        nc.scalar.dma_start(out=bvr[:], in_=ins["bvr"].broadcast_to([128, D]))
        b2r = _mk(po, [128, D], BF, "b2r")
        nc.scalar.dma_start(out=b2r[:], in_=ins["b2r"].broadcast_to([128, D]))
        lnw = {}
        for nm in ("g1", "be1", "g2", "be2"):
            t = _mk(po, [128, D], BF, "ln_" + nm)
            nc.scalar.dma_start(out=t[:], in_=ins[nm].broadcast_to([128, D]))
            lnw[nm] = t
        eps = _mk(po, [128, 1], F32, "eps")
        nc.vector.memset(eps[:], 1e-5)
        lnw["eps"] = eps
        ones_f = _mk(po, [128, 64], F32, "ones_f")
        nc.vector.memset(ones_f[:], 1.0)
        xn = [_mk(po, [128, D], BF, f"xn{r}") for r in range(4)]
        for r in range(4):
            nc.scalar.dma_start(out=xn[r][:],
                                in_=ins["xn"][r * 128:(r + 1) * 128, :])

        # persistent post-phase activations
        x1n = [_mk(po, [128, D], BF, f"x1n{r}") for r in range(4)]
        x1nb = [_mk(po, [128, D], BF, f"x1nb{r}") for r in range(4)]
        x1nT = [_mk(po, [128, RPC], BF, f"x1nT{dc}") for dc in range(8)]

        with tc.tile_pool(name="attn", bufs=1) as pa:
            _attn_phase(nc, tc, ins, po, pa, pp, ident, bias, bvh, bvr, b2r,
                        xn, x1n, x1nb, x1nT, lnw, ones_f)

        with tc.tile_pool(name="post", bufs=1) as pf:
            _post_phase(nc, tc, ins, pf, pp, bias, lnw, x1n, x1nb, x1nT)


def _attn_phase(nc, tc, ins, po, pa, pp, ident, bias, bvh, bvr, b2r,
                xn, x1n, x1nb, x1nT, lnw, ones_f):
    oT = [_mk(pa, [128, RPC], BF, f"oT{p}") for p in range(NP)]
    kTs, qTs, vqs = {}, {}, {}

    with tc.tile_pool(name="proj", bufs=1) as px:
        xT = [_mk(px, [128, FB], BF, f"xT{dc}") for dc in range(8)]
        for dc in range(8):
            nc.sync.dma_start(out=xT[dc][:],
                              in_=ins["xbT"][dc * 128:(dc + 1) * 128, :])
        xoT = [_mk(px, [128, RPC], BF, f"xoT{dc}") for dc in range(8)]
        for dc in range(8):
            nc.scalar.dma_start(out=xoT[dc][:],
                                in_=ins["xoT"][dc * 128:(dc + 1) * 128, :])

        def proj_items(p):
            """(pair, unit, thunk) items, sorted by deadline."""
            st = {}

            def dma_w():
                wkt = _mk(px, [128, 1024], BF, "wkt", bufs=2)
                nc.sync.dma_start(out=wkt[:], in_=ins["wk"][p])
                wqt = _mk(px, [128, 1024], BF, "wqt", bufs=2)
                nc.sync.dma_start(out=wqt[:], in_=ins["wq"][p])
                kTs[p] = _mk(pa, [128, FB], BF, "kT", bufs=3)
                qTs[p] = _mk(pa, [128, RPC], BF, "qT", bufs=3)
                st["wkt"], st["wqt"] = wkt, wqt

            def k_chunk(c):
                ps = _mk(pp, [128, 512], F32, "ps_p", bufs=2)
                for dc in range(8):
                    nc.tensor.matmul(ps[:],
                                     st["wkt"][:, dc * 128:(dc + 1) * 128],
                                     xT[dc][:, c * 512:(c + 1) * 512],
                                     start=(dc == 0), stop=(dc == 7),
                                     skip_group_check=True)
                nc.vector.tensor_scalar(kTs[p][:, c * 512:(c + 1) * 512],
                                        ps[:], bias["bk"][:, p:p + 1],
                                        None, OP.add)

            def q_chunk():
                ps = _mk(pp, [128, 512], F32, "ps_p", bufs=2)
                for dc in range(8):
                    nc.tensor.matmul(ps[:],
                                     st["wqt"][:, dc * 128:(dc + 1) * 128],
                                     xoT[dc][:], start=(dc == 0), stop=(dc == 7),
                                     skip_group_check=True)
                nc.vector.tensor_scalar(qTs[p][:], ps[:],
                                        bias["bq"][:, p:p + 1], None, OP.add)

            items = [(p, 0, dma_w), (p, 0, lambda: k_chunk(0)), (p, 0, q_chunk)]
            for c in range(1, 4):
                items.append((p, 2 * c, lambda c=c: k_chunk(c)))

            if p % 4 == 0:
                hf = p // 4

                def dma_v():
                    wvt = _mk(px, [128, 4096], BF, "wvt", bufs=1)
                    nc.sync.dma_start(out=wvt[:], in_=ins["wv"][hf])
                    vq = _mk(pa, [128, 4 * 2 * NKT * 65], BF, "vq", bufs=2)
                    vqs[hf] = vq
                    vqv = vq[:].rearrange("k (i h t c) -> k i h t c",
                                          i=4, h=2, c=65)
                    for pl in range(4):
                        nc.vector.memset(vqv[:, pl, :, :, 64:65], 1.0)
                    st["wvt"] = wvt

                def v_chunk(kt2):
                    vqv = vqs[hf][:].rearrange("k (i h t c) -> k i h t c",
                                               i=4, h=2, c=65)
                    bvs = bvr[:, hf * 512:(hf + 1) * 512].rearrange(
                        "k (i h c) -> k i h c", i=4, c=64)
                    ps = _mk(pp, [128, 512], F32, "ps_p", bufs=2)
                    for dc in range(8):
                        nc.tensor.matmul(ps[:],
                                         xT[dc][:, kt2 * 128:(kt2 + 1) * 128],
                                         st["wvt"][:, dc * 512:(dc + 1) * 512],
                                         start=(dc == 0), stop=(dc == 7),
                                         skip_group_check=True)
                    nc.vector.tensor_tensor(
                        vqv[:, :, :, kt2, 0:64],
                        ps[:].rearrange("k (i h c) -> k i h c", i=4, c=64),
                        bvs, OP.add)

                items.append((p, 0, dma_v))
                for kt2 in range(NKT):
                    items.append((p, kt2 // 2, lambda kt2=kt2: v_chunk(kt2)))

            items.sort(key=lambda it: (it[0], it[1]))
            return items

        def attn_unit(p, u, pso):
            kt, qt, vq = kTs[p], qTs[p], vqs[p // 4]
            pl = p % 4
            vqv = vq[:].rearrange("k (i h t c) -> k i h t c", i=4, h=2, c=65)
            pss = [_mk(pp, [128, 1024], F32, "ps_s", bufs=2) for _ in range(2)]
            for i in range(2):
                ktile = 2 * u + i
                for hh in range(2):
                    nc.tensor.matmul(
                        pss[hh][:, i * 512:(i + 1) * 512],
                        kt[hh * 64:(hh + 1) * 64,
                           ktile * 128:(ktile + 1) * 128],
                        qt[hh * 64:(hh + 1) * 64, :],
                        start=True, stop=True, skip_group_check=True)
            pts = []
            for hh in range(2):
                pt = _mk(pa, [128, 1024], BF, "pt", bufs=4)
                nc.scalar.activation(pt[:], pss[hh][:], AF.Exp,
                                     bias=0.0, scale=0.125)
                pts.append(pt)
            for i in range(2):
                ktile = 2 * u + i
                for hh in range(2):
                    nc.tensor.matmul(
                        pso[hh][0:65, :],
                        vqv[:, pl, hh, ktile, :],
                        pts[hh][:, i * 512:(i + 1) * 512],
                        start=(ktile == 0), stop=(ktile == NKT - 1),
                        skip_group_check=True)

        def attn_norm_pre(p, pso):
            """1/den = exp(-ln den), on the scalar engine (off PE/DVE)."""
            rdens = []
            for hh in range(2):
                nden = _mk(pa, [128, 512], F32, "den", bufs=1)
                nc.scalar.activation(nden[64:65, :], pso[hh][64:65, :],
                                     AF.Ln, bias=0.0, scale=1.0)
                rden = _mk(pa, [128, 512], F32, "rden", bufs=2)
                nc.scalar.activation(rden[64:65, :], nden[64:65, :],
                                     AF.Exp, bias=0.0, scale=-1.0)
                rdens.append(rden)
            return rdens

        def attn_norm_post(p, pso, rdens):
            for hh in range(2):
                h = 2 * p + hh
                ps_b = _mk(pp, [128, 512], F32, "ps_p", bufs=2)
                nc.tensor.matmul(ps_b[0:64, :], ones_f[64:65, :],
                                 rdens[hh][64:65, :], start=True, stop=True,
                                 skip_group_check=True)
                rb = _mk(pa, [128, 512], F32, "rb", bufs=1)
                nc.vector.tensor_copy(rb[0:64, :], ps_b[0:64, :])
                tmp = _mk(pa, [128, 512], F32, "onorm", bufs=1)
                nc.vector.tensor_tensor(tmp[0:64, :], pso[hh][0:64, :],
                                        rb[0:64, :], OP.mult)
                if hh == 0:
                    nc.vector.tensor_scalar(oT[p][0:64, :], tmp[0:64, :],
                                            bvh[0:64, h:h + 1], None, OP.add)
                else:
                    stage = _mk(pa, [128, 512], BF, "stage", bufs=2)
                    nc.vector.tensor_scalar(stage[0:64, :], tmp[0:64, :],
                                            bvh[0:64, h:h + 1], None, OP.add)
                    nc.gpsimd.dma_start(out=oT[p][64:128, :],
                                        in_=stage[0:64, :])

        # ---- deadline-scheduled weave (proj runs up to 2 pairs ahead;
        # kT/qT tag bufs=3 require never enqueueing more than p+2) ----
        pending = deque()
        for pre in range(min(2, NP)):
            pending.extend(proj_items(pre))

        def drain(n):
            for _ in range(min(n, len(pending))):
                pending.popleft()[2]()

        def drain_due(p, u):
            while pending and (pending[0][0], pending[0][1]) <= (p, u):
                pending.popleft()[2]()

        for p in range(NP):
            if p + 2 < NP:
                pending.extend(proj_items(p + 2))
            pso = [_mk(pp, [128, 512], F32, "ps_o", bufs=2) for _ in range(2)]
            for u in range(NKT // 2):
                drain_due(p, u)
                attn_unit(p, u, pso)
                drain(2)
            rdens = attn_norm_pre(p, pso)
            drain(1)
            attn_norm_post(p, pso, rdens)
        drain(len(pending))

    # ---- Wo in natural orientation: mha = sum_dc oT_chunk.T @ Wo_rows ----
    # Stationary (oT chunk) is reused across both output halves; output
    # needs no transpose. x1 = mha + (x + bo)  [bo folded into xn host-side]
    wot = [_mk(pa, [128, 1024], BF, f"wot{dc}") for dc in range(8)]
    for dc in range(8):
        nc.sync.dma_start(out=wot[dc][:], in_=ins["wo"][dc])
    pst = [_mk(pp, [128, 2048], BF, "ps_s", bufs=2) for _ in range(2)]
    for r in range(4):
        ps = [_mk(pp, [128, 512], F32, "ps_p", bufs=2) for _ in range(2)]
        for dc in range(8):
            for hf in range(2):
                nc.tensor.matmul(ps[hf][:],
                                 oT[dc][:, r * 128:(r + 1) * 128],
                                 wot[dc][:, hf * 512:(hf + 1) * 512],
                                 start=(dc == 0), stop=(dc == 7),
                                 skip_group_check=True)
        x1 = _mk(po, [128, D], BF, "x1", bufs=2)
        for hf in range(2):
            nc.vector.tensor_tensor(x1[:, hf * 512:(hf + 1) * 512],
                                    ps[hf][:], xn[r][:, hf * 512:(hf + 1) * 512],
                                    OP.add)
        _layernorm(nc, po, x1n[r], x1, lnw["g1"], lnw["be1"], lnw["eps"])
        nc.vector.tensor_tensor(x1nb[r][:], x1n[r][:], b2r[:], OP.add)
        for dc in range(8):
            nc.tensor.transpose(
                pst[dc // 4][:, (dc % 4) * 512 + r * 128:
                             (dc % 4) * 512 + (r + 1) * 128],
                x1n[r][:, dc * 128:(dc + 1) * 128], ident[:])
    for dc in range(8):
        nc.vector.tensor_copy(x1nT[dc][:],
                              pst[dc // 4][:, (dc % 4) * 512:
                                           (dc % 4 + 1) * 512])


def _post_phase(nc, tc, ins, pf, pp, bias, lnw, x1n, x1nb, x1nT):
    # ---- FFN1: hT[f] = relu(W1_f.T @ x1nT + b1) ----
    w2t = [_mk(pf, [128, 1024], BF, f"w2t{f}") for f in range(32)]
    hT = [_mk(pf, [128, RPC], BF, f"hT{f}") for f in range(32)]
    for f in range(32):
        w1t = _mk(pf, [128, 1024], BF, "w1t", bufs=4)
        nc.sync.dma_start(out=w1t[:], in_=ins["w1"][f])
        nc.sync.dma_start(out=w2t[f][:], in_=ins["w2"][f])
        ps = _mk(pp, [128, 512], F32, "ps_p", bufs=2)
        for dc in range(8):
            nc.tensor.matmul(ps[:], w1t[:, dc * 128:(dc + 1) * 128],
                             x1nT[dc][:], start=(dc == 0), stop=(dc == 7),
                             skip_group_check=True)
        nc.scalar.activation(hT[f][:], ps[:], AF.Relu,
                             bias=bias["b1"][:, f:f + 1], scale=1.0)

    # ---- FFN2 in natural orientation + residual + LN2 + out ----
    for r in range(4):
        ps = [_mk(pp, [128, 512], F32, "ps_p", bufs=2) for _ in range(2)]
        for f in range(32):
            for hf in range(2):
                nc.tensor.matmul(ps[hf][:],
                                 hT[f][:, r * 128:(r + 1) * 128],
                                 w2t[f][:, hf * 512:(hf + 1) * 512],
                                 start=(f == 0), stop=(f == 31),
                                 skip_group_check=True)
        x2 = _mk(pf, [128, D], BF, "x2", bufs=2)
        for hf in range(2):
            nc.vector.tensor_tensor(x2[:, hf * 512:(hf + 1) * 512], ps[hf][:],
                                    x1nb[r][:, hf * 512:(hf + 1) * 512],
                                    OP.add)
        outt = _mk(pf, [128, D], F32, "outt", bufs=2)
        _layernorm(nc, pf, outt, x2, lnw["g2"], lnw["be2"], lnw["eps"])
        nc.sync.dma_start(out=ins["out"][r * 128:(r + 1) * 128, :],
                          in_=outt[:])


def _layernorm(nc, pool, out, x, g, be, eps):
    """LN along the free dim (D=1024). x [128, 1024] bf16; out bf16/f32."""
    _TCNT[0] += 1
    n = _TCNT[0]
    stats = pool.tile([128, 2, 6], F32, tag="ln_st", bufs=2, name=f"lnst{n}")
    for i in range(2):
        nc.vector.bn_stats(stats[:, i, :], x[:, i * 512:(i + 1) * 512])
    mv = pool.tile([128, 2], F32, tag="ln_mv", bufs=2, name=f"lnmv{n}")
    nc.vector.bn_aggr(mv[:], stats[:])
    std = pool.tile([128, 1], F32, tag="ln_sd", bufs=2, name=f"lnsd{n}")
    nc.scalar.activation(std[:], mv[:, 1:2], AF.Sqrt, bias=eps, scale=1.0)
    rstd = pool.tile([128, 1], F32, tag="ln_rs", bufs=2, name=f"lnrs{n}")
    nc.vector.reciprocal(rstd[:], std[:])
    t = pool.tile([128, D], BF, tag="ln_t", bufs=2, name=f"lnt{n}")
    nc.vector.tensor_scalar(t[:], x[:], mv[:, 0:1], rstd[:],
                            OP.subtract, OP.mult)
    t2 = pool.tile([128, D], BF, tag="ln_t2", bufs=2, name=f"lnt2{n}")
    nc.vector.tensor_tensor(t2[:], t[:], g[:], OP.mult)
    nc.vector.tensor_tensor(out[:], t2[:], be[:], OP.add)


def prep_inputs(x, Wq, bq, Wk, bk, Wv, bv, Wo, bo, W1, b1, W2, b2,
                g1, be1, g2, be2):
    """Host-side prep: per-core inputs, weights pre-cast to bf16.

    Stationary-weight layouts are [*, 128, n] with the 128 SBUF
    partitions contiguous-major so each tile is one dense DMA.
    bo is folded into the xn residual input.
    """
    import ml_dtypes
    f = np.float32
    bf = ml_dtypes.bfloat16

    def _qdc(w, ncol):  # [D_in, ncols] -> [ncols/ncol, 128, 8*ncol]
        # element (blk, q, dc*ncol+c) = w[dc*128+q, blk*ncol+c]
        nblk = w.shape[1] // ncol
        return np.ascontiguousarray(
            np.asarray(w, f).reshape(8, 128, nblk, ncol).transpose(2, 1, 0, 3)
            .reshape(nblk, 128, 8 * ncol)).astype(bf)

    wq_flat = np.asarray(Wq, f).transpose(1, 0, 2).reshape(D, D)
    wk_flat = np.asarray(Wk, f).transpose(1, 0, 2).reshape(D, D)
    wv_flat = np.asarray(Wv, f).transpose(1, 0, 2).reshape(D, D)
    common = {
        "wq": _qdc(wq_flat, 128), "wk": _qdc(wk_flat, 128),
        "wv": _qdc(wv_flat, 512),
        "wo": np.asarray(Wo, f).reshape(8, 128, D).astype(bf),
        "w1": _qdc(np.asarray(W1, f), 128),
        "w2": np.asarray(W2, f).reshape(32, 128, D).astype(bf),
        "bq": np.asarray(bq, f).reshape(D, 1),
        "bk": np.asarray(bk, f).reshape(D, 1),
        "bv": np.asarray(bv, f).reshape(D, 1),
        "bvr": np.asarray(bv, f).reshape(1, D).astype(bf),
        "b1": np.asarray(b1, f).reshape(DFF, 1),
        "b2r": np.asarray(b2, f).reshape(1, D).astype(bf),
        "g1": np.asarray(g1, f).reshape(1, D).astype(bf),
        "be1": np.asarray(be1, f).reshape(1, D).astype(bf),
        "g2": np.asarray(g2, f).reshape(1, D).astype(bf),
        "be2": np.asarray(be2, f).reshape(1, D).astype(bf),
        "ident": np.eye(128, dtype=f).astype(bf),
    }
    xf = np.asarray(x, f)
    bo_f = np.asarray(bo, f).reshape(1, D)
    xbT = [np.ascontiguousarray(xf[b].T).astype(bf) for b in range(B)]
    in_maps = []
    for c in range(NCORES):
        b, j = divmod(c, 4)
        m = dict(common)
        m["xbT"] = xbT[b]
        own = xf[b, j * RPC:(j + 1) * RPC, :]
        m["xoT"] = np.ascontiguousarray(own.T).astype(bf)
        m["xn"] = np.ascontiguousarray(own + bo_f).astype(bf)
        in_maps.append(m)
    return in_maps


_NC_CACHE = {}
LAST_EXEC_NS = None
LAST_TRACE_PATH = None
LAST_PROFILE_JSON = None


def kernel(**inputs) -> np.ndarray:
    global LAST_EXEC_NS, LAST_TRACE_PATH, LAST_PROFILE_JSON
    if "main" not in _NC_CACHE:
        _NC_CACHE["main"] = build_nc()
    nc = _NC_CACHE["main"]
    in_maps = prep_inputs(**inputs)
    res = run_bass_kernel_spmd(nc, in_maps, core_ids=list(range(NCORES)))
    LAST_EXEC_NS = getattr(res, "exec_time_ns", None)
    LAST_PROFILE_JSON = getattr(res, "profile_json", None)
    it = getattr(res, "instructions_and_trace", None)
    LAST_TRACE_PATH = it[1] if it else None
    out = np.empty((B, S, D), np.float32)
    for c in range(NCORES):
        b, j = divmod(c, 4)
        out[b, j * RPC:(j + 1) * RPC, :] = res.results[c]["out"]
    return out


# revision 33
# speedup vs baseline: 1.2381x; 1.0110x over previous
"""Trainium2 Bass kernel for nn_Encoder (dense transformer encoder layer).

Sharding: 8 NeuronCores, sequence-parallel, zero collectives. B*S = 4096
rows -> 512 rows per core; cores 0-3 handle batch 0, cores 4-7 batch 1.
The full batch's x (transposed, bf16) is replicated to every core as an
input, so each core computes K^T and V for the WHOLE batch locally (no
K/V AllGather), then runs attention for its own 512 query rows over all
16 heads, plus Wo / LN1 / FFN / LN2 locally.

All matmuls bf16 (full PE rate, weights pre-cast host-side); psum
accumulation fp32. End-to-end relative error ~1e-2 (budget 2e-2).

Schedule: every projection chunk (K/Q/V) carries a (pair, unit)
deadline; chunks are emitted just-in-time between attention units so
the PE never idles long enough to re-throttle while the scalar engine
streams the softmax exp() ops. Attention starts ~10us into the kernel.

Dataflow:
  xT [8dc x 128, 2048] bf16 (host-transposed full-batch input)
  kT[p] [128, 2048] = Wk_p.T @ xT  (full batch keys, feature-major)
  vq[half]: V columns for 4 pairs, keys on partition, ones column per
            (pair,hh,keytile) for the softmax denominator
  qT[p] [128, 512] from xoT (own rows, host-transposed)
  S^T [128 keys, 512 q] = kts.T @ qT  (2 heads packed at rows 0/64)
  P = exp(0.125 * S^T) -> bf16, one ACT op per 2 key tiles
  O^T[65,512] += [V|1].T @ P  (psum row 64 = denominator)
  oT = O^T * bcast(exp(-ln den)) + bv
  mha (natural) = sum_dc oT_chunk.T @ Wo_rows  (stationary reuse, no
      output transpose); x1 = mha + (x + bo)   [bo folded host-side]
  LN1 -> x1n -> x1nT;  hT = relu(W1.T @ x1nT + b1)
  ffn (natural) = sum_f hT_chunk.T @ W2_rows; x2 = ffn + x1n + b2
  LN2 -> out [512, 1024] f32
"""

from collections import deque

import numpy as np

import concourse.bass as bass
import concourse.mybir as mybir
from concourse import bacc
from concourse.tile import TileContext
from concourse.bass_utils import run_bass_kernel_spmd

F32 = mybir.dt.float32
BF = mybir.dt.bfloat16
AF = mybir.ActivationFunctionType
OP = mybir.AluOpType

B, S, D = 2, 2048, 1024
H, DK, DFF = 16, 64, 4096
NCORES = 8
RPC = S * B // NCORES          # 512 own rows per core
FB = S                         # 2048 full-batch rows
NP = H // 2                    # 8 head pairs
NKT = FB // 128                # 16 key tiles

_TCNT = [0]


def _mk(pool, shape, dt, tag, bufs=None):
    _TCNT[0] += 1
    kw = {} if bufs is None else {"bufs": bufs}
    return pool.tile(shape, dt, tag=tag, name=f"t{_TCNT[0]}_{tag}", **kw)


def build_nc():
    nc = bacc.Bacc(num_devices=NCORES)

    ins = dict(
        xbT=nc.dram_tensor("xbT", [D, FB], BF, kind="ExternalInput"),
        xoT=nc.dram_tensor("xoT", [D, RPC], BF, kind="ExternalInput"),
        xn=nc.dram_tensor("xn", [RPC, D], BF, kind="ExternalInput"),
        wq=nc.dram_tensor("wq", [NP, 128, 1024], BF, kind="ExternalInput"),
        wk=nc.dram_tensor("wk", [NP, 128, 1024], BF, kind="ExternalInput"),
        wv=nc.dram_tensor("wv", [2, 128, 4096], BF, kind="ExternalInput"),
        wo=nc.dram_tensor("wo", [8, 128, 1024], BF, kind="ExternalInput"),
        w1=nc.dram_tensor("w1", [32, 128, 1024], BF, kind="ExternalInput"),
        w2=nc.dram_tensor("w2", [32, 128, 1024], BF, kind="ExternalInput"),
        bq=nc.dram_tensor("bq", [D, 1], F32, kind="ExternalInput"),
        bk=nc.dram_tensor("bk", [D, 1], F32, kind="ExternalInput"),
        bv=nc.dram_tensor("bv", [D, 1], F32, kind="ExternalInput"),
        bvr=nc.dram_tensor("bvr", [1, D], BF, kind="ExternalInput"),
        b1=nc.dram_tensor("b1", [DFF, 1], F32, kind="ExternalInput"),
        b2r=nc.dram_tensor("b2r", [1, D], BF, kind="ExternalInput"),
        g1=nc.dram_tensor("g1", [1, D], BF, kind="ExternalInput"),
        be1=nc.dram_tensor("be1", [1, D], BF, kind="ExternalInput"),
        g2=nc.dram_tensor("g2", [1, D], BF, kind="ExternalInput"),
        be2=nc.dram_tensor("be2", [1, D], BF, kind="ExternalInput"),
        ident=nc.dram_tensor("ident", [128, 128], BF, kind="ExternalInput"),
        out=nc.dram_tensor("out", [RPC, D], F32, kind="ExternalOutput"),
    )

    with TileContext(nc) as tc:
        _body(nc, tc, ins)

    nc.finalize()
    return nc


def _body(nc, tc, ins):
    with (
        tc.tile_pool(name="outer", bufs=1) as po,
        tc.tile_pool(name="psum", bufs=1, space="PSUM") as pp,
    ):
        # ---- constants ----
        ident = _mk(po, [128, 128], BF, "ident")
        nc.scalar.dma_start(out=ident[:], in_=ins["ident"][:])
        bias = {}
        for nm, n in (("bq", 8), ("bk", 8), ("b1", 32)):
            t = _mk(po, [128, n], F32, "b_" + nm)
            nc.scalar.dma_start(out=t[:],
                                in_=ins[nm].rearrange("(i p) o -> p (i o)", p=128))
            bias[nm] = t
        # bv in per-head layout: col h = bv[h*64:(h+1)*64] on partitions 0-63
        bvh = _mk(po, [128, H], F32, "bvh")
        nc.scalar.dma_start(out=bvh[0:64, :],
                            in_=ins["bv"].rearrange("(h p) o -> p (h o)", p=64))
        bvr = _mk(po, [128, D], BF, "bvr")
        nc.scalar.dma_start(out=bvr[:], in_=ins["bvr"].broadcast_to([128, D]))
        b2r = _mk(po, [128, D], BF, "b2r")
        nc.scalar.dma_start(out=b2r[:], in_=ins["b2r"].broadcast_to([128, D]))
        lnw = {}
        for nm in ("g1", "be1", "g2", "be2"):
            t = _mk(po, [128, D], BF, "ln_" + nm)
            nc.scalar.dma_start(out=t[:], in_=ins[nm].broadcast_to([128, D]))
            lnw[nm] = t
        eps = _mk(po, [128, 1], F32, "eps")
        nc.vector.memset(eps[:], 1e-5)
        lnw["eps"] = eps
        ones_f = _mk(po, [128, 64], F32, "ones_f")
        nc.vector.memset(ones_f[:], 1.0)
        xn = [_mk(po, [128, D], BF, f"xn{r}") for r in range(4)]
        for r in range(4):
            nc.scalar.dma_start(out=xn[r][:],
                                in_=ins["xn"][r * 128:(r + 1) * 128, :])

        # persistent post-phase activations
        x1n = [_mk(po, [128, D], BF, f"x1n{r}") for r in range(4)]
        x1nb = [_mk(po, [128, D], BF, f"x1nb{r}") for r in range(4)]
        x1nT = [_mk(po, [128, RPC], BF, f"x1nT{dc}") for dc in range(8)]

        with tc.tile_pool(name="attn", bufs=1) as pa:
            _attn_phase(nc, tc, ins, po, pa, pp, ident, bias, bvh, bvr, b2r,
                        xn, x1n, x1nb, x1nT, lnw, ones_f)

        with tc.tile_pool(name="post", bufs=1) as pf:
            _post_phase(nc, tc, ins, pf, pp, bias, lnw, x1n, x1nb, x1nT)


def _attn_phase(nc, tc, ins, po, pa, pp, ident, bias, bvh, bvr, b2r,
                xn, x1n, x1nb, x1nT, lnw, ones_f):
    oT = [_mk(pa, [128, RPC], BF, f"oT{p}") for p in range(NP)]
    kTs, qTs, vqs = {}, {}, {}

    with tc.tile_pool(name="proj", bufs=1) as px:
        xT = [_mk(px, [128, FB], BF, f"xT{dc}") for dc in range(8)]
        for dc in range(8):
            nc.sync.dma_start(out=xT[dc][:],
                              in_=ins["xbT"][dc * 128:(dc + 1) * 128, :])
        xoT = [_mk(px, [128, RPC], BF, f"xoT{dc}") for dc in range(8)]
        for dc in range(8):
            nc.scalar.dma_start(out=xoT[dc][:],
                                in_=ins["xoT"][dc * 128:(dc + 1) * 128, :])

        def proj_items(p):
            """(pair, unit, thunk) items, sorted by deadline."""
            st = {}

            def dma_w():
                wkt = _mk(px, [128, 1024], BF, "wkt", bufs=2)
                nc.sync.dma_start(out=wkt[:], in_=ins["wk"][p])
                wqt = _mk(px, [128, 1024], BF, "wqt", bufs=2)
                nc.sync.dma_start(out=wqt[:], in_=ins["wq"][p])
                kTs[p] = _mk(pa, [128, FB], BF, "kT", bufs=3)
                qTs[p] = _mk(pa, [128, RPC], BF, "qT", bufs=3)
                st["wkt"], st["wqt"] = wkt, wqt

            def k_chunk(c):
                ps = _mk(pp, [128, 512], F32, "ps_p", bufs=2)
                for dc in range(8):
                    nc.tensor.matmul(ps[:],
                                     st["wkt"][:, dc * 128:(dc + 1) * 128],
                                     xT[dc][:, c * 512:(c + 1) * 512],
                                     start=(dc == 0), stop=(dc == 7),
                                     skip_group_check=True)
                nc.vector.tensor_scalar(kTs[p][:, c * 512:(c + 1) * 512],
                                        ps[:], bias["bk"][:, p:p + 1],
                                        None, OP.add)

            def q_chunk():
                ps = _mk(pp, [128, 512], F32, "ps_p", bufs=2)
                for dc in range(8):
                    nc.tensor.matmul(ps[:],
                                     st["wqt"][:, dc * 128:(dc + 1) * 128],
                                     xoT[dc][:], start=(dc == 0), stop=(dc == 7),
                                     skip_group_check=True)
                nc.vector.tensor_scalar(qTs[p][:], ps[:],
                                        bias["bq"][:, p:p + 1], None, OP.add)

            items = [(p, 0, dma_w), (p, 0, lambda: k_chunk(0)), (p, 0, q_chunk)]
            for c in range(1, 4):
                items.append((p, 2 * c, lambda c=c: k_chunk(c)))

            if p % 4 == 0:
                hf = p // 4

                def dma_v():
                    wvt = _mk(px, [128, 4096], BF, "wvt", bufs=1)
                    nc.sync.dma_start(out=wvt[:], in_=ins["wv"][hf])
                    vq = _mk(pa, [128, 4 * 2 * NKT * 65], BF, "vq", bufs=2)
                    vqs[hf] = vq
                    vqv = vq[:].rearrange("k (i h t c) -> k i h t c",
                                          i=4, h=2, c=65)
                    for pl in range(4):
                        nc.vector.memset(vqv[:, pl, :, :, 64:65], 1.0)
                    st["wvt"] = wvt

                def v_chunk(kt2):
                    vqv = vqs[hf][:].rearrange("k (i h t c) -> k i h t c",
                                               i=4, h=2, c=65)
                    bvs = bvr[:, hf * 512:(hf + 1) * 512].rearrange(
                        "k (i h c) -> k i h c", i=4, c=64)
                    ps = _mk(pp, [128, 512], F32, "ps_p", bufs=2)
                    for dc in range(8):
                        nc.tensor.matmul(ps[:],
                                         xT[dc][:, kt2 * 128:(kt2 + 1) * 128],
                                         st["wvt"][:, dc * 512:(dc + 1) * 512],
                                         start=(dc == 0), stop=(dc == 7),
                                         skip_group_check=True)
                    nc.vector.tensor_tensor(
                        vqv[:, :, :, kt2, 0:64],
                        ps[:].rearrange("k (i h c) -> k i h c", i=4, c=64),
                        bvs, OP.add)

                items.append((p, 0, dma_v))
                for kt2 in range(NKT):
                    items.append((p, kt2 // 2, lambda kt2=kt2: v_chunk(kt2)))

            items.sort(key=lambda it: (it[0], it[1]))
            return items

        def attn_unit(p, u, pso):
            kt, qt, vq = kTs[p], qTs[p], vqs[p // 4]
            pl = p % 4
            vqv = vq[:].rearrange("k (i h t c) -> k i h t c", i=4, h=2, c=65)
            pss = [_mk(pp, [128, 1024], F32, "ps_s", bufs=2) for _ in range(2)]
            for i in range(2):
                ktile = 2 * u + i
                for hh in range(2):
                    nc.tensor.matmul(
                        pss[hh][:, i * 512:(i + 1) * 512],
                        kt[hh * 64:(hh + 1) * 64,
                           ktile * 128:(ktile + 1) * 128],
                        qt[hh * 64:(hh + 1) * 64, :],
                        start=True, stop=True, skip_group_check=True)
            pts = []
            for hh in range(2):
                pt = _mk(pa, [128, 1024], BF, "pt", bufs=3)
                nc.scalar.activation(pt[:], pss[hh][:], AF.Exp,
                                     bias=0.0, scale=0.125)
                pts.append(pt)
            for i in range(2):
                ktile = 2 * u + i
                for hh in range(2):
                    nc.tensor.matmul(
                        pso[hh][0:65, :],
                        vqv[:, pl, hh, ktile, :],
                        pts[hh][:, i * 512:(i + 1) * 512],
                        start=(ktile == 0), stop=(ktile == NKT - 1),
                        skip_group_check=True)

        def attn_stash(p, pso):
            """Copy raw O and den off PSUM at pair end (frees the banks);
            normalization is deferred into the next pair's stream."""
            st = []
            for hh in range(2):
                o_raw = _mk(pa, [128, 512], BF, "o_raw", bufs=2)
                nc.vector.tensor_copy(o_raw[0:64, :], pso[hh][0:64, :])
                den = _mk(pa, [128, 512], BF, "dens", bufs=2)
                nc.vector.tensor_copy(den[64:65, :], pso[hh][64:65, :])
                st.append((o_raw, den))
            return st

        def attn_normalize(p, st):
            for hh in range(2):
                h = 2 * p + hh
                o_raw, den = st[hh]
                rden = _mk(pa, [128, 512], F32, "rden", bufs=2)
                nc.vector.reciprocal(rden[64:65, :], den[64:65, :])
                ps_b = _mk(pp, [128, 512], F32, "ps_p", bufs=2)
                nc.tensor.matmul(ps_b[0:64, :], ones_f[64:65, :],
                                 rden[64:65, :], start=True, stop=True,
                                 skip_group_check=True)
                rb = _mk(pa, [128, 512], F32, "rb", bufs=1)
                nc.vector.tensor_copy(rb[0:64, :], ps_b[0:64, :])
                tmp = _mk(pa, [128, 512], F32, "onorm", bufs=1)
                nc.vector.tensor_tensor(tmp[0:64, :], o_raw[0:64, :],
                                        rb[0:64, :], OP.mult)
                if hh == 0:
                    nc.vector.tensor_scalar(oT[p][0:64, :], tmp[0:64, :],
                                            bvh[0:64, h:h + 1], None, OP.add)
                else:
                    stage = _mk(pa, [128, 512], BF, "stage", bufs=1)
                    nc.vector.tensor_scalar(stage[0:64, :], tmp[0:64, :],
                                            bvh[0:64, h:h + 1], None, OP.add)
                    nc.gpsimd.dma_start(out=oT[p][64:128, :],
                                        in_=stage[0:64, :])

        # ---- deadline-scheduled weave (proj runs up to 2 pairs ahead;
        # kT/qT tag bufs=3 require never enqueueing more than p+2) ----
        pending = deque()
        for pre in range(min(2, NP)):
            pending.extend(proj_items(pre))

        def drain(n):
            for _ in range(min(n, len(pending))):
                pending.popleft()[2]()

        def drain_due(p, u):
            while pending and (pending[0][0], pending[0][1]) <= (p, u):
                pending.popleft()[2]()

        stash = None
        for p in range(NP):
            if p + 2 < NP:
                pending.extend(proj_items(p + 2))
            pso = [_mk(pp, [128, 512], F32, "ps_o", bufs=2) for _ in range(2)]
            for u in range(NKT // 2):
                drain_due(p, u)
                attn_unit(p, u, pso)
                if u == 1 and stash is not None:
                    attn_normalize(p - 1, stash)
                drain(2)
            stash = attn_stash(p, pso)
        attn_normalize(NP - 1, stash)
        drain(len(pending))

    # ---- Wo in natural orientation: mha = sum_dc oT_chunk.T @ Wo_rows ----
    # Stationary (oT chunk) is reused across both output halves; output
    # needs no transpose. x1 = mha + (x + bo)  [bo folded into xn host-side]
    wot = [_mk(pa, [128, 1024], BF, f"wot{dc}") for dc in range(8)]
    for dc in range(8):
        nc.sync.dma_start(out=wot[dc][:], in_=ins["wo"][dc])
    pst = [_mk(pp, [128, 2048], BF, "ps_s", bufs=2) for _ in range(2)]
    for r in range(4):
        ps = [_mk(pp, [128, 512], F32, "ps_p", bufs=2) for _ in range(2)]
        for dc in range(8):
            for hf in range(2):
                nc.tensor.matmul(ps[hf][:],
                                 oT[dc][:, r * 128:(r + 1) * 128],
                                 wot[dc][:, hf * 512:(hf + 1) * 512],
                                 start=(dc == 0), stop=(dc == 7),
                                 skip_group_check=True)
        x1 = _mk(po, [128, D], BF, "x1", bufs=2)
        for hf in range(2):
            nc.vector.tensor_tensor(x1[:, hf * 512:(hf + 1) * 512],
                                    ps[hf][:], xn[r][:, hf * 512:(hf + 1) * 512],
                                    OP.add)
        _layernorm(nc, po, x1n[r], x1, lnw["g1"], lnw["be1"], lnw["eps"])
        nc.vector.tensor_tensor(x1nb[r][:], x1n[r][:], b2r[:], OP.add)
        for dc in range(8):
            nc.tensor.transpose(
                pst[dc // 4][:, (dc % 4) * 512 + r * 128:
                             (dc % 4) * 512 + (r + 1) * 128],
                x1n[r][:, dc * 128:(dc + 1) * 128], ident[:])
    for dc in range(8):
        nc.vector.tensor_copy(x1nT[dc][:],
                              pst[dc // 4][:, (dc % 4) * 512:
                                           (dc % 4 + 1) * 512])


def _post_phase(nc, tc, ins, pf, pp, bias, lnw, x1n, x1nb, x1nT):
    # ---- FFN1: hT[f] = relu(W1_f.T @ x1nT + b1) ----
    w2t = [_mk(pf, [128, 1024], BF, f"w2t{f}") for f in range(32)]
    hT = [_mk(pf, [128, RPC], BF, f"hT{f}") for f in range(32)]
    for f in range(32):
        w1t = _mk(pf, [128, 1024], BF, "w1t", bufs=4)
        nc.sync.dma_start(out=w1t[:], in_=ins["w1"][f])
        nc.sync.dma_start(out=w2t[f][:], in_=ins["w2"][f])
        ps = _mk(pp, [128, 512], F32, "ps_p", bufs=2)
        for dc in range(8):
            nc.tensor.matmul(ps[:], w1t[:, dc * 128:(dc + 1) * 128],
                             x1nT[dc][:], start=(dc == 0), stop=(dc == 7),
                             skip_group_check=True)
        nc.scalar.activation(hT[f][:], ps[:], AF.Relu,
                             bias=bias["b1"][:, f:f + 1], scale=1.0)

    # ---- FFN2 in natural orientation + residual + LN2 + out ----
    for r in range(4):
        ps = [_mk(pp, [128, 512], F32, "ps_p", bufs=2) for _ in range(2)]
        for f in range(32):
            for hf in range(2):
                nc.tensor.matmul(ps[hf][:],
                                 hT[f][:, r * 128:(r + 1) * 128],
                                 w2t[f][:, hf * 512:(hf + 1) * 512],
                                 start=(f == 0), stop=(f == 31),
                                 skip_group_check=True)
        x2 = _mk(pf, [128, D], BF, "x2", bufs=2)
        for hf in range(2):
            nc.vector.tensor_tensor(x2[:, hf * 512:(hf + 1) * 512], ps[hf][:],
                                    x1nb[r][:, hf * 512:(hf + 1) * 512],
                                    OP.add)
        outt = _mk(pf, [128, D], F32, "outt", bufs=2)
        _layernorm(nc, pf, outt, x2, lnw["g2"], lnw["be2"], lnw["eps"])
        nc.sync.dma_start(out=ins["out"][r * 128:(r + 1) * 128, :],
                          in_=outt[:])


def _layernorm(nc, pool, out, x, g, be, eps):
    """LN along the free dim (D=1024). x [128, 1024] bf16; out bf16/f32."""
    _TCNT[0] += 1
    n = _TCNT[0]
    stats = pool.tile([128, 2, 6], F32, tag="ln_st", bufs=2, name=f"lnst{n}")
    for i in range(2):
        nc.vector.bn_stats(stats[:, i, :], x[:, i * 512:(i + 1) * 512])
    mv = pool.tile([128, 2], F32, tag="ln_mv", bufs=2, name=f"lnmv{n}")
    nc.vector.bn_aggr(mv[:], stats[:])
    std = pool.tile([128, 1], F32, tag="ln_sd", bufs=2, name=f"lnsd{n}")
    nc.scalar.activation(std[:], mv[:, 1:2], AF.Sqrt, bias=eps, scale=1.0)
    rstd = pool.tile([128, 1], F32, tag="ln_rs", bufs=2, name=f"lnrs{n}")
    nc.vector.reciprocal(rstd[:], std[:])
    t = pool.tile([128, D], BF, tag="ln_t", bufs=2, name=f"lnt{n}")
    nc.vector.tensor_scalar(t[:], x[:], mv[:, 0:1], rstd[:],
                            OP.subtract, OP.mult)
    t2 = pool.tile([128, D], BF, tag="ln_t2", bufs=2, name=f"lnt2{n}")
    nc.vector.tensor_tensor(t2[:], t[:], g[:], OP.mult)
    nc.vector.tensor_tensor(out[:], t2[:], be[:], OP.add)


def prep_inputs(x, Wq, bq, Wk, bk, Wv, bv, Wo, bo, W1, b1, W2, b2,
                g1, be1, g2, be2):
    """Host-side prep: per-core inputs, weights pre-cast to bf16.

    Stationary-weight layouts are [*, 128, n] with the 128 SBUF
    partitions contiguous-major so each tile is one dense DMA.
    bo is folded into the xn residual input.
    """
    import ml_dtypes
    f = np.float32
    bf = ml_dtypes.bfloat16

    def _qdc(w, ncol):  # [D_in, ncols] -> [ncols/ncol, 128, 8*ncol]
        # element (blk, q, dc*ncol+c) = w[dc*128+q, blk*ncol+c]
        nblk = w.shape[1] // ncol
        return np.ascontiguousarray(
            np.asarray(w, f).reshape(8, 128, nblk, ncol).transpose(2, 1, 0, 3)
            .reshape(nblk, 128, 8 * ncol)).astype(bf)

    wq_flat = np.asarray(Wq, f).transpose(1, 0, 2).reshape(D, D)
    wk_flat = np.asarray(Wk, f).transpose(1, 0, 2).reshape(D, D)
    wv_flat = np.asarray(Wv, f).transpose(1, 0, 2).reshape(D, D)
    common = {
        "wq": _qdc(wq_flat, 128), "wk": _qdc(wk_flat, 128),
        "wv": _qdc(wv_flat, 512),
        "wo": np.asarray(Wo, f).reshape(8, 128, D).astype(bf),
        "w1": _qdc(np.asarray(W1, f), 128),
        "w2": np.asarray(W2, f).reshape(32, 128, D).astype(bf),
        "bq": np.asarray(bq, f).reshape(D, 1),
        "bk": np.asarray(bk, f).reshape(D, 1),
        "bv": np.asarray(bv, f).reshape(D, 1),
        "bvr": np.asarray(bv, f).reshape(1, D).astype(bf),
        "b1": np.asarray(b1, f).reshape(DFF, 1),
        "b2r": np.asarray(b2, f).reshape(1, D).astype(bf),
        "g1": np.asarray(g1, f).reshape(1, D).astype(bf),
        "be1": np.asarray(be1, f).reshape(1, D).astype(bf),
        "g2": np.asarray(g2, f).reshape(1, D).astype(bf),
        "be2": np.asarray(be2, f).reshape(1, D).astype(bf),
        "ident": np.eye(128, dtype=f).astype(bf),
    }
    xf = np.asarray(x, f)
    bo_f = np.asarray(bo, f).reshape(1, D)
    xbT = [np.ascontiguousarray(xf[b].T).astype(bf) for b in range(B)]
    in_maps = []
    for c in range(NCORES):
        b, j = divmod(c, 4)
        m = dict(common)
        m["xbT"] = xbT[b]
        own = xf[b, j * RPC:(j + 1) * RPC, :]
        m["xoT"] = np.ascontiguousarray(own.T).astype(bf)
        m["xn"] = np.ascontiguousarray(own + bo_f).astype(bf)
        in_maps.append(m)
    return in_maps


_NC_CACHE = {}
LAST_EXEC_NS = None
LAST_TRACE_PATH = None
LAST_PROFILE_JSON = None


def kernel(**inputs) -> np.ndarray:
    global LAST_EXEC_NS, LAST_TRACE_PATH, LAST_PROFILE_JSON
    if "main" not in _NC_CACHE:
        _NC_CACHE["main"] = build_nc()
    nc = _NC_CACHE["main"]
    in_maps = prep_inputs(**inputs)
    res = run_bass_kernel_spmd(nc, in_maps, core_ids=list(range(NCORES)))
    LAST_EXEC_NS = getattr(res, "exec_time_ns", None)
    LAST_PROFILE_JSON = getattr(res, "profile_json", None)
    it = getattr(res, "instructions_and_trace", None)
    LAST_TRACE_PATH = it[1] if it else None
    out = np.empty((B, S, D), np.float32)
    for c in range(NCORES):
        b, j = divmod(c, 4)
        out[b, j * RPC:(j + 1) * RPC, :] = res.results[c]["out"]
    return out
